# revision 14
# baseline (speedup 1.0000x reference)
"""3-layer LSTM decoder (T=256, B=1024, H=64/128/1) with locked dropout.

Data-parallel over batch: B=1024 -> 128 per core x 8 NeuronCores.
Single fused Bass/Tile kernel per core runs all three layer scans as a
wavefront (iteration tau computes L1 step tau, L2 step tau-1, L3 step
tau-2), v3: all matmul operands bf16 (FWL weight loads, no fp32
decomposition), skinny L3 matmuls, and the iteration split into two
independent chains (L2 and L1+L3) each with its own PSUM, gate ACT,
cell triplet and S ACT so the Tile scheduler pipelines them across
engines.  G/S stay fp32 in SBUF: the (sigma-0.5) terms cancel near 0
and bf16 storage there is what blew up v2's error.

Math: tanh(x) = 2*sigma(2x)-1; cell tracked as C=c/2, hidden H=h/2,
g-gate weights pre-scaled by 2, H-consuming weights pre-scaled by 2.
  u = (sg-0.5)*si ; v = sf*C ; C = v+u ; S = sigma(4C) ; H = (S-0.5)*so
Locked dropout folded as f2 = H1*m1, f3 = H2*m2 (DVE); mask3 and the
2x for h3 are applied on the host during the gather.
"""

import os
import sys

sys.path.insert(0, "/opt/trn_rl_repo/concourse")
sys.path.insert(0, "/opt/trn_rl_repo")

import ml_dtypes
import numpy as np

import concourse.bass as bass
import concourse.mybir as mybir
import concourse.tile as tile
import bass_rust
from concourse.tile_sem_assignment import N_PROCS

T, B, NCORES = 256, 1024, 8
BC = B // NCORES          # batch per core
H1, H2 = 64, 128
F32 = mybir.dt.float32
BF16 = mybir.dt.bfloat16
SIG = mybir.ActivationFunctionType.Sigmoid

# slot s in [i, f, o, g] order -> row-block index in torch [i, f, g, o] weights
TG = [0, 1, 3, 2]

LAST_RESULTS = None  # BassKernelResults of the most recent run (for test.py)


# ---------------------------------------------------------------- tile patch
def _patched_drain_and_barrier(self, tick_clock, wait_clock):
    # This walrus build rejects instructions carrying more than one sem
    # wait ("Too many sync wait commands") and TileContext's stock tail
    # drain carries one wait per outstanding proc.  Spread them over one
    # SP NoOp per proc; SP program order then makes the drain itself safe
    # with no waits.
    nc = self.nc
    gclock = tick_clock.global_clock
    for p in range(N_PROCS):
        if gclock[p] <= 0:
            continue
        partial = bass_rust.VectorClock()
        partial.require_at_least(p, gclock[p])
        nop = nc.sync.nop(nofuse=True, hint=f"tile_tail_wait_p{p}")
        wait_clock.add_sem_waits(nop.ins, bass_rust.ScopedClock({None: partial}))
    nc.sync.drain()
    nc.all_engine_barrier()
    assert self.sems is not None
    popped = nc._tile_sem_poison_stack.pop()
    assert popped is self._sem_poison
    nc.clear_and_free_semaphores(list(self.sems.allocated().values()))
    nc.all_engine_barrier()


tile.TileContext._drain_and_barrier = _patched_drain_and_barrier


# ---------------------------------------------------------------- builder
def build(t_steps=T):
    """Build the SPMD single-core Bass program for t_steps timesteps."""
    nc = bass.Bass("TRN2", target_bir_lowering=False, debug=False)

    # xt blocks of 8 steps: [blk, 0, j*BC:(j+1)*BC] = x_{8blk+j}; [blk,1,:] = 1
    nblk_x = (t_steps + 7) // 8
    xt = nc.declare_dram_parameter("xt", [nblk_x, 2, 8 * BC], BF16, isOutput=False)
    w13 = nc.declare_dram_parameter("w13", [98, 512], BF16, isOutput=False)
    w2rec = nc.declare_dram_parameter("w2rec", [128, 512], BF16, isOutput=False)
    w2fold = nc.declare_dram_parameter("w2fold", [65, 512], BF16, isOutput=False)
    w3sk = nc.declare_dram_parameter("w3sk", [128, 4], BF16, isOutput=False)
    m1t = nc.declare_dram_parameter("m1t", [H1, BC], BF16, isOutput=False)
    m2t = nc.declare_dram_parameter("m2t", [H2, BC], BF16, isOutput=False)
    n_out = (t_steps + 2) // 8 + 2
    h3st = nc.declare_dram_parameter("h3st", [n_out, 8 * BC], BF16, isOutput=True)

    nblk = (t_steps + 7) // 8
    with tile.TileContext(nc) as tc:
        with (
            tc.tile_pool(name="const", bufs=1) as cpool,
            tc.tile_pool(name="ring", bufs=1) as ring,
            tc.tile_pool(name="work", bufs=2) as work,
            tc.tile_pool(name="psum", bufs=2, space="PSUM") as pp,
            tc.tile_pool(name="junk", bufs=1, space="PSUM") as jp,
        ):
            # -------- constants
            w13_t = cpool.tile([98, 512], BF16, name="w13_t")
            nc.gpsimd.dma_start(w13_t[:], w13[:])
            w2r_t = cpool.tile([128, 512], BF16, name="w2r_t")
            nc.gpsimd.dma_start(w2r_t[:], w2rec[:])
            w2f_t = cpool.tile([65, 512], BF16, name="w2f_t")
            nc.gpsimd.dma_start(w2f_t[:], w2fold[:])
            w3s_t = cpool.tile([128, 4], BF16, name="w3s_t")
            nc.gpsimd.dma_start(w3s_t[:], w3sk[:])
            m1_t = cpool.tile([H1, BC], BF16, name="m1_t")
            nc.gpsimd.dma_start(m1_t[:], m1t[:])
            m2_t = cpool.tile([H2, BC], BF16, name="m2_t")
            nc.gpsimd.dma_start(m2_t[:], m2t[:])

            # -------- state
            # 16-slot ring; slot tau%16 is iter tau's L13 matmul rhs.
            # rows 0-63 H1, row 64 H3, 65-95 zero, 96 x_t, 97 ones.
            rg = ring.tile([98, 16 * BC], BF16, name="rg")
            h2b = [ring.tile([128, BC], BF16, name=f"h2b_{i}") for i in range(2)]
            l2f = [ring.tile([65, BC], BF16, name=f"l2f_{i}") for i in range(2)]
            l3f = [ring.tile([128, BC], BF16, name=f"l3f_{i}") for i in range(2)]
            # cell state: block 0 = C2 (rows 0:128), block 1 = C13 (rows 0:65)
            Ct = ring.tile([128, 2, 128], F32, name="Ct")
            # PE_HAM un-throttles the PE clock (1.2 -> 2.4 GHz) only while
            # the PE looks busy; the real matmul bursts are too short, so
            # the array runs cold.  jmm() issues a 1-column junk matmul
            # gated on a mid-chain tile, spreading PE activity through the
            # ACT/DVE phase to keep the activity monitor high.
            junk = jp.tile([128, 8], F32, name="junk")
            jw = cpool.tile([128, 1], F32, name="jw")
            nc.vector.memset(jw[:], 0.0)

            def jmm(rhs):
                """rhs: any ready fp32 [128, 8] slice; result is discarded."""
                nc.tensor.matmul(
                    junk[0:1, 0:8], jw[:], rhs,
                    start=True, stop=True, skip_group_check=True,
                )

            nc.vector.memset(rg[0:96, :], 0.0)
            nc.vector.memset(rg[96:98, :], 1.0)
            for j in range(2):
                nc.vector.memset(h2b[j][:], 0.0)
                nc.vector.memset(l2f[j][0:64, :], 0.0)
                nc.vector.memset(l2f[j][64:65, :], 1.0)
                nc.vector.memset(l3f[j][:], 0.0)
            nc.vector.memset(Ct[:], 0.0)
            # x+ones for iters [0,8) and [8,16)
            for blk in range(min(2, nblk)):
                nc.sync.dma_start(
                    rg[96:98, blk * 8 * BC : (blk + 1) * 8 * BC], xt[blk]
                )

            out_row = 0
            # -------- main wavefront loop
            for tau in range(t_steps + 2):
                sl = (tau % 16) * BC
                so = ((tau + 1) % 16) * BC
                hcur, hnext = h2b[tau % 2], h2b[(tau + 1) % 2]
                f2cur, f2next = l2f[tau % 2], l2f[(tau + 1) % 2]
                f3cur, f3next = l3f[tau % 2], l3f[(tau + 1) % 2]

                if tau % 8 == 0 and tau > 0 and tau + 8 < t_steps:
                    blk, half = (tau + 8) // 8, (((tau + 8) % 16) // 8)
                    nc.sync.dma_start(
                        rg[96:98, half * 8 * BC : (half + 1) * 8 * BC], xt[blk]
                    )

                # Two independent chains per iteration (the wavefront makes
                # L2's step and L1/L3's step data-independent): each gets its
                # own PSUM, gate ACT, cell triplet and S ACT so the Tile
                # scheduler can overlap them across engines.
                # PSUM layout per group: 4 slots of 128: [i|f|o|g].
                ps2 = pp.tile([128, 4, 128], F32, name="ps2")
                for s in range(4):
                    c = s * 128
                    nc.tensor.matmul(
                        ps2[:, s, :],
                        w2r_t[:, c : c + 128], hcur[:],
                        start=True, stop=False,
                    )
                    nc.tensor.matmul(
                        ps2[:, s, :],
                        w2f_t[:, c : c + 128], f2cur[:],
                        start=False, stop=True,
                    )
                G2 = work.tile([128, 4, 128], F32, name="G2")
                nc.scalar.activation(
                    G2[:].rearrange("p a b -> p (a b)"),
                    ps2[:].rearrange("p a b -> p (a b)"), SIG,
                )
                jmm(G2[:, 0, 0:8])

                ps3 = pp.tile([128, 4, 128], F32, name="ps3")
                for s in range(4):
                    nc.tensor.matmul(
                        ps3[:, s, :],
                        w13_t[:, s * 128 : (s + 1) * 128], rg[0:98, sl : sl + BC],
                        start=True, stop=False,
                    )
                    nc.tensor.matmul(
                        ps3[64:65, s, :],
                        w3s_t[:, s : s + 1], f3cur[:],
                        start=False, stop=True, skip_group_check=True,
                    )
                G3 = work.tile([128, 4, 128], F32, name="G3")
                nc.scalar.activation(
                    G3[:].rearrange("p a b -> p (a b)"),
                    ps3[:].rearrange("p a b -> p (a b)"), SIG,
                )
                jmm(G3[:, 0, 0:8])

                # ---- L2 cell chain
                u2 = work.tile([128, 128], F32, name="u2")
                nc.vector.scalar_tensor_tensor(
                    u2[:], G2[:, 3, :], 0.5, G2[:, 0, :],
                    mybir.AluOpType.subtract, mybir.AluOpType.mult,
                )
                v2 = work.tile([128, 128], F32, name="v2")
                nc.vector.tensor_mul(v2[:], G2[:, 1, :], Ct[:, 0, :])
                nc.vector.tensor_add(Ct[:, 0, :], v2[:], u2[:])
                jmm(v2[:, 0:8])
                S2 = work.tile([128, 128], F32, name="S2")
                nc.scalar.activation(S2[:], Ct[:, 0, :], SIG, scale=4.0)
                jmm(S2[:, 0:8])
                nc.vector.scalar_tensor_tensor(
                    hnext[:], S2[:], 0.5, G2[:, 2, :],
                    mybir.AluOpType.subtract, mybir.AluOpType.mult,
                )

                # ---- L13 cell chain
                u3 = work.tile([128, 128], F32, name="u3")
                nc.vector.scalar_tensor_tensor(
                    u3[:], G3[:, 3, :], 0.5, G3[:, 0, :],
                    mybir.AluOpType.subtract, mybir.AluOpType.mult,
                )
                v3 = work.tile([128, 128], F32, name="v3")
                nc.vector.tensor_mul(v3[:], G3[:, 1, :], Ct[:, 1, :])
                nc.vector.tensor_add(Ct[:, 1, :], v3[:], u3[:])
                jmm(v3[:, 0:8])
                S3 = work.tile([128, 128], F32, name="S3")
                nc.scalar.activation(S3[:], Ct[:, 1, :], SIG, scale=4.0)
                jmm(S3[:, 0:8])
                nc.vector.scalar_tensor_tensor(
                    rg[0:65, so : so + BC], S3[0:65, :], 0.5, G3[0:65, 2, :],
                    mybir.AluOpType.subtract, mybir.AluOpType.mult,
                )

                # ---- dropout folds (consumed next iteration: full slack)
                nc.gpsimd.tensor_mul(f3next[:], hnext[:], m2_t[:])
                nc.gpsimd.tensor_mul(f2next[0:64, :], rg[0:64, so : so + BC], m1_t[:])

                if tau % 8 == 6:
                    half = (((tau + 1) % 16) - 7) // 8
                    nc.sync.dma_start(
                        h3st[out_row : out_row + 1, :],
                        rg[64:65, half * 8 * BC : (half + 1) * 8 * BC],
                    )
                    out_row += 1

                # boundary fix-ups: wipe garbage states before first real use
                if tau == 0:
                    nc.vector.memset(Ct[:, 0, :], 0.0)            # C2
                    nc.vector.memset(h2b[1][:], 0.0)              # H2
                if tau == 1:
                    nc.vector.memset(Ct[64:65, 1, :], 0.0)        # C3
                    nc.vector.memset(rg[64:65, 2 * BC : 3 * BC], 0.0)  # H3 slot 2

            # final flush: both halves (tail slots depend on t_steps % 16)
            for half in range(2):
                nc.sync.dma_start(
                    h3st[out_row : out_row + 1, :],
                    rg[64:65, half * 8 * BC : (half + 1) * 8 * BC],
                )
                out_row += 1

    return nc


# ---------------------------------------------------------------- host prep
def pack_weights(Wih1, Whh1, b1, Wih2, Whh2, b2, Wih3, Whh3, b3):
    """Pack/scale weights into the kernel's lhsT layouts (see module doc)."""
    w13 = np.zeros((98, 512), np.float32)
    w2rec = np.zeros((128, 512), np.float32)
    w2fold = np.zeros((65, 512), np.float32)
    w3sk = np.zeros((128, 4), np.float32)
    for s in range(4):
        tg = TG[s]
        gs = 2.0 if s == 3 else 1.0  # sigma(2x) pre-scale for the g slot
        c = s * 128
        # L1 block: rows 0-63 = 2*Whh1^T, row 96 = Wih1, row 97 = b1
        w13[0:64, c : c + 64] = 2.0 * gs * Whh1[tg * 64 : (tg + 1) * 64, :].T
        w13[96, c : c + 64] = gs * Wih1[tg * 64 : (tg + 1) * 64, 0]
        w13[97, c : c + 64] = gs * b1[tg * 64 : (tg + 1) * 64]
        # L3 col 64: row 64 = 2*Whh3, row 97 = b3
        w13[64, c + 64] = 2.0 * gs * Whh3[tg, 0]
        w13[97, c + 64] = gs * b3[tg]
        w3sk[:, s] = 2.0 * gs * Wih3[tg, :]
        # L2
        w2rec[:, c : c + 128] = 2.0 * gs * Whh2[tg * 128 : (tg + 1) * 128, :].T
        w2fold[0:64, c : c + 128] = 2.0 * gs * Wih2[tg * 128 : (tg + 1) * 128, :].T
        w2fold[64, c : c + 128] = gs * b2[tg * 128 : (tg + 1) * 128]
    return dict(w13=w13, w2rec=w2rec, w2fold=w2fold, w3sk=w3sk)


def make_in_maps(inputs, t_steps=T):
    bf = ml_dtypes.bfloat16
    w = pack_weights(
        inputs["Wih1"], inputs["Whh1"], inputs["b1"],
        inputs["Wih2"], inputs["Whh2"], inputs["b2"],
        inputs["Wih3"], inputs["Whh3"], inputs["b3"],
    )
    w = {k: v.astype(bf) for k, v in w.items()}
    x = np.asarray(inputs["x"], np.float32)
    m1 = np.asarray(inputs["mask1"], np.float32)
    m2 = np.asarray(inputs["mask2"], np.float32)
    in_maps = []
    for c in range(NCORES):
        sl = slice(c * BC, (c + 1) * BC)
        nblk = (t_steps + 7) // 8
        xa = np.zeros((nblk, 2, 8 * BC), np.float32)
        xc = x[:t_steps, sl, 0]  # [t_steps, BC]
        for blk in range(nblk):
            n = min(8, t_steps - blk * 8)
            xa[blk, 0, : n * BC] = xc[blk * 8 : blk * 8 + n].reshape(-1)
        xa[:, 1, :] = 1.0
        in_maps.append({
            "xt": xa.astype(bf),
            "m1t": np.ascontiguousarray(m1[sl, :].T).astype(bf),
            "m2t": np.ascontiguousarray(m2[sl, :].T).astype(bf),
            **{k: v for k, v in w.items()},
        })
    return in_maps


def _split_multi_waits(bir):
    """This walrus build allows at most ONE sem wait per instruction.

    Tile's scheduler attaches as many waits as deps require, so split:
    any instruction with k>1 waits gets k-1 single-wait NoOps inserted
    before it on the same engine (sequencer order preserves semantics)."""
    n = 0
    for f in bir.get("functions", []):
        for bb in f.get("basic_blocks", f.get("blocks", [])):
            insts = bb.get("instructions", [])
            out = []
            for inst in insts:
                si = inst.get("sync_info")
                waits = (si or {}).get("on_wait") or []
                if len(waits) > 1:
                    for w in waits[:-1]:
                        n += 1
                        out.append({
                            "debug": inst.get("debug", 0),
                            "engine": inst["engine"],
                            "ins": [],
                            "name": f"WSPLIT-{n}",
                            "opcode": "NoOp",
                            "outs": [],
                            "sync_info": {"on_update": [], "on_wait": [w]},
                            "text_hint": "wait_split",
                        })
                    si["on_wait"] = [waits[-1]]
                out.append(inst)
            bb["instructions"] = out
    return n


def finalize(nc):
    """Apply the multi-wait split to nc's serialized BIR (idempotent)."""
    import orjson

    if getattr(nc, "_wsplit_done", False):
        return nc
    bir = orjson.loads(nc.to_json_bytes())
    n = _split_multi_waits(bir)
    blob = orjson.dumps(bir)
    nc.to_json_bytes = lambda: blob
    nc._wsplit_done = True
    nc._wsplit_count = n
    return nc


def out_schedule(t_steps=T):
    """Replay the out-DMA emission schedule.

    Returns a list (one entry per h3st row) of 8-tuples: the LSTM step
    whose H3 occupies slot j of that row (-1 if junk)."""
    last_write = [None] * 16     # slot -> iter of last H13 write
    rows = []
    for tau in range(t_steps + 2):
        last_write[(tau + 1) % 16] = tau
        if tau % 8 == 6:
            half = (((tau + 1) % 16) - 7) // 8
            rows.append(tuple(
                (last_write[8 * half + j] - 2)
                if last_write[8 * half + j] is not None else -1
                for j in range(8)
            ))
    for half in range(2):
        rows.append(tuple(
            (last_write[8 * half + j] - 2)
            if last_write[8 * half + j] is not None else -1
            for j in range(8)
        ))
    return rows


_BUILT = {}


def kernel(**inputs) -> np.ndarray:
    global LAST_RESULTS
    from concourse.bass_utils import run_bass_kernel_spmd

    if T not in _BUILT:
        _BUILT[T] = finalize(build(T))
    nc = _BUILT[T]
    in_maps = make_in_maps(inputs, T)
    res = run_bass_kernel_spmd(
        nc, in_maps, list(range(NCORES)),
        trace=bool(os.environ.get("BASS_TRACE")),
    )
    LAST_RESULTS = res
    m3 = np.asarray(inputs["mask3"], np.float32)  # [B, 1]
    sched = out_schedule(T)
    out = np.empty((T, B, 1), np.float32)
    for c in range(NCORES):
        sl = slice(c * BC, (c + 1) * BC)
        h3 = np.asarray(res.results[c]["h3st"], dtype=np.float32)  # [n_out, 8*BC]
        dec = np.empty((T, BC), np.float32)
        for r, steps in enumerate(sched):
            for j, st in enumerate(steps):
                if 0 <= st < T:
                    dec[st] = h3[r, j * BC : (j + 1) * BC]
        # h3 = 2*H3; output = h3 * mask3
        out[:, sl, 0] = 2.0 * dec * m3[sl, 0][None, :]
    return out


# revision 17
# speedup vs baseline: 1.1696x; 1.1696x over previous
"""3-layer LSTM decoder (T=256, B=1024, H=64/128/1) with locked dropout.

Data-parallel over batch: B=1024 -> 128 per core x 8 NeuronCores.
Single fused Bass/Tile kernel per core runs all three layer scans as a
wavefront (iteration tau computes L1 step tau, L2 step tau-2, L3 step
tau-4 -- the extra lag gives the dropout-fold tiles >1 iteration of
slack so the two chains never wait on each other's tail), v5: all matmul operands bf16 (FWL weight loads, no fp32
decomposition), skinny L3 matmuls, and the iteration split into two
independent chains (L2 and L1+L3) each with its own PSUM, gate ACT,
cell triplet and S ACT so the Tile scheduler pipelines them across
engines.  G/S stay fp32 in SBUF: the (sigma-0.5) terms cancel near 0
and bf16 storage there is what blew up v2's error.

Math: tanh(x) = 2*sigma(2x)-1; cell tracked as C=c/2, hidden H=h/2,
g-gate weights pre-scaled by 2, H-consuming weights pre-scaled by 2.
  u = (sg-0.5)*si ; v = sf*C ; C = v+u ; S = sigma(4C) ; H = (S-0.5)*so
Locked dropout folded as f2 = H1*m1, f3 = H2*m2 (DVE); mask3 and the
2x for h3 are applied on the host during the gather.
"""

import os
import sys

sys.path.insert(0, "/opt/trn_rl_repo/concourse")
sys.path.insert(0, "/opt/trn_rl_repo")

import ml_dtypes
import numpy as np

import concourse.bass as bass
import concourse.mybir as mybir
import concourse.tile as tile
import bass_rust
from concourse.tile_sem_assignment import N_PROCS

T, B, NCORES = 256, 1024, 8
BC = B // NCORES          # batch per core
H1, H2 = 64, 128
F32 = mybir.dt.float32
BF16 = mybir.dt.bfloat16
SIG = mybir.ActivationFunctionType.Sigmoid

# slot s in [i, f, o, g] order -> row-block index in torch [i, f, g, o] weights
TG = [0, 1, 3, 2]

LAST_RESULTS = None  # BassKernelResults of the most recent run (for test.py)


# ---------------------------------------------------------------- tile patch
def _patched_drain_and_barrier(self, tick_clock, wait_clock):
    # This walrus build rejects instructions carrying more than one sem
    # wait ("Too many sync wait commands") and TileContext's stock tail
    # drain carries one wait per outstanding proc.  Spread them over one
    # SP NoOp per proc; SP program order then makes the drain itself safe
    # with no waits.
    nc = self.nc
    gclock = tick_clock.global_clock
    for p in range(N_PROCS):
        if gclock[p] <= 0:
            continue
        partial = bass_rust.VectorClock()
        partial.require_at_least(p, gclock[p])
        nop = nc.sync.nop(nofuse=True, hint=f"tile_tail_wait_p{p}")
        wait_clock.add_sem_waits(nop.ins, bass_rust.ScopedClock({None: partial}))
    nc.sync.drain()
    nc.all_engine_barrier()
    assert self.sems is not None
    popped = nc._tile_sem_poison_stack.pop()
    assert popped is self._sem_poison
    nc.clear_and_free_semaphores(list(self.sems.allocated().values()))
    nc.all_engine_barrier()


tile.TileContext._drain_and_barrier = _patched_drain_and_barrier


# ---------------------------------------------------------------- builder
def build(t_steps=T):
    """Build the SPMD single-core Bass program for t_steps timesteps."""
    nc = bass.Bass("TRN2", target_bir_lowering=False, debug=False)

    # xt blocks of 8 steps: [blk, 0, j*BC:(j+1)*BC] = x_{8blk+j}; [blk,1,:] = 1
    nblk_x = (t_steps + 7) // 8
    xt = nc.declare_dram_parameter("xt", [nblk_x, 2, 8 * BC], BF16, isOutput=False)
    w13 = nc.declare_dram_parameter("w13", [98, 512], BF16, isOutput=False)
    w2rec = nc.declare_dram_parameter("w2rec", [128, 512], BF16, isOutput=False)
    w2fold = nc.declare_dram_parameter("w2fold", [65, 512], BF16, isOutput=False)
    w3sk = nc.declare_dram_parameter("w3sk", [128, 4], BF16, isOutput=False)
    m1t = nc.declare_dram_parameter("m1t", [H1, BC], BF16, isOutput=False)
    m2t = nc.declare_dram_parameter("m2t", [H2, BC], BF16, isOutput=False)
    n_out = (t_steps + 4) // 8 + 2
    h3st = nc.declare_dram_parameter("h3st", [n_out, 8 * BC], BF16, isOutput=True)

    nblk = (t_steps + 7) // 8
    with tile.TileContext(nc) as tc:
        with (
            tc.tile_pool(name="const", bufs=1) as cpool,
            tc.tile_pool(name="ring", bufs=1) as ring,
            tc.tile_pool(name="work", bufs=2) as work,
            tc.tile_pool(name="psum", bufs=2, space="PSUM") as pp,
        ):
            # -------- constants
            w13_t = cpool.tile([98, 512], BF16, name="w13_t")
            nc.gpsimd.dma_start(w13_t[:], w13[:])
            w2r_t = cpool.tile([128, 512], BF16, name="w2r_t")
            nc.gpsimd.dma_start(w2r_t[:], w2rec[:])
            w2f_t = cpool.tile([65, 512], BF16, name="w2f_t")
            nc.gpsimd.dma_start(w2f_t[:], w2fold[:])
            w3s_t = cpool.tile([128, 4], BF16, name="w3s_t")
            nc.gpsimd.dma_start(w3s_t[:], w3sk[:])
            m1_t = cpool.tile([H1, BC], BF16, name="m1_t")
            nc.gpsimd.dma_start(m1_t[:], m1t[:])
            m2_t = cpool.tile([H2, BC], BF16, name="m2_t")
            nc.gpsimd.dma_start(m2_t[:], m2t[:])

            # -------- state
            # 16-slot ring; slot tau%16 is iter tau's L13 matmul rhs.
            # rows 0-63 H1, row 64 H3, 65-95 zero, 96 x_t, 97 ones.
            rg = ring.tile([98, 16 * BC], BF16, name="rg")
            h2b = [ring.tile([128, BC], BF16, name=f"h2b_{i}") for i in range(2)]
            l2f = [ring.tile([65, BC], BF16, name=f"l2f_{i}") for i in range(3)]
            l3f = [ring.tile([128, BC], BF16, name=f"l3f_{i}") for i in range(3)]
            # cell state: block 0 = C2 (rows 0:128), block 1 = C13 (rows 0:65)
            Ct = ring.tile([128, 2, 128], F32, name="Ct")

            nc.vector.memset(rg[0:96, :], 0.0)
            nc.vector.memset(rg[96:98, :], 1.0)
            for j in range(2):
                nc.vector.memset(h2b[j][:], 0.0)
            for j in range(3):
                nc.vector.memset(l2f[j][0:64, :], 0.0)
                nc.vector.memset(l2f[j][64:65, :], 1.0)
                nc.vector.memset(l3f[j][:], 0.0)
            nc.vector.memset(Ct[:], 0.0)
            # x+ones for iters [0,8) and [8,16)
            for blk in range(min(2, nblk)):
                nc.sync.dma_start(
                    rg[96:98, blk * 8 * BC : (blk + 1) * 8 * BC], xt[blk]
                )

            out_row = 0
            # -------- main wavefront loop
            for tau in range(t_steps + 4):
                sl = (tau % 16) * BC
                so = ((tau + 1) % 16) * BC
                hcur, hnext = h2b[tau % 2], h2b[(tau + 1) % 2]
                f2cur, f2next = l2f[(tau + 1) % 3], l2f[tau % 3]
                f3cur, f3next = l3f[(tau + 1) % 3], l3f[tau % 3]

                if tau % 8 == 0 and tau > 0 and tau + 8 < t_steps:
                    blk, half = (tau + 8) // 8, (((tau + 8) % 16) // 8)
                    nc.sync.dma_start(
                        rg[96:98, half * 8 * BC : (half + 1) * 8 * BC], xt[blk]
                    )

                # Two independent chains per iteration (the wavefront makes
                # L2's step and L1/L3's step data-independent): each gets its
                # own PSUM, gate ACT, cell triplet and S ACT so the Tile
                # scheduler can overlap them across engines.
                # PSUM layout per group: 4 slots of 128: [i|f|o|g].
                ps2 = pp.tile([128, 4, 128], F32, name="ps2")
                for s in range(4):
                    c = s * 128
                    nc.tensor.matmul(
                        ps2[:, s, :],
                        w2r_t[:, c : c + 128], hcur[:],
                        start=True, stop=False,
                    )
                    nc.tensor.matmul(
                        ps2[:, s, :],
                        w2f_t[:, c : c + 128], f2cur[:],
                        start=False, stop=True,
                    )
                G2 = work.tile([128, 4, 128], F32, name="G2")
                nc.scalar.activation(
                    G2[:].rearrange("p a b -> p (a b)"),
                    ps2[:].rearrange("p a b -> p (a b)"), SIG,
                )

                ps3 = pp.tile([128, 4, 128], F32, name="ps3")
                for s in range(4):
                    nc.tensor.matmul(
                        ps3[:, s, :],
                        w13_t[:, s * 128 : (s + 1) * 128], rg[0:98, sl : sl + BC],
                        start=True, stop=False,
                    )
                    nc.tensor.matmul(
                        ps3[64:65, s, :],
                        w3s_t[:, s : s + 1], f3cur[:],
                        start=False, stop=True, skip_group_check=True,
                    )
                G3 = work.tile([128, 4, 128], F32, name="G3")
                nc.scalar.activation(
                    G3[:].rearrange("p a b -> p (a b)"),
                    ps3[:].rearrange("p a b -> p (a b)"), SIG,
                )

                # ---- L2 cell chain
                u2 = work.tile([128, 128], F32, name="u2")
                nc.vector.scalar_tensor_tensor(
                    u2[:], G2[:, 3, :], 0.5, G2[:, 0, :],
                    mybir.AluOpType.subtract, mybir.AluOpType.mult,
                )
                v2 = work.tile([128, 128], F32, name="v2")
                nc.vector.tensor_mul(v2[:], G2[:, 1, :], Ct[:, 0, :])
                nc.vector.tensor_add(Ct[:, 0, :], v2[:], u2[:])
                S2 = work.tile([128, 128], F32, name="S2")
                nc.scalar.activation(S2[:], Ct[:, 0, :], SIG, scale=4.0)
                nc.vector.scalar_tensor_tensor(
                    hnext[:], S2[:], 0.5, G2[:, 2, :],
                    mybir.AluOpType.subtract, mybir.AluOpType.mult,
                )

                # ---- L13 cell chain
                u3 = work.tile([128, 128], F32, name="u3")
                nc.vector.scalar_tensor_tensor(
                    u3[:], G3[:, 3, :], 0.5, G3[:, 0, :],
                    mybir.AluOpType.subtract, mybir.AluOpType.mult,
                )
                v3 = work.tile([128, 128], F32, name="v3")
                nc.vector.tensor_mul(v3[:], G3[:, 1, :], Ct[:, 1, :])
                nc.vector.tensor_add(Ct[:, 1, :], v3[:], u3[:])
                S3 = work.tile([128, 128], F32, name="S3")
                nc.scalar.activation(S3[:], Ct[:, 1, :], SIG, scale=4.0)
                nc.vector.scalar_tensor_tensor(
                    rg[0:65, so : so + BC], S3[0:65, :], 0.5, G3[0:65, 2, :],
                    mybir.AluOpType.subtract, mybir.AluOpType.mult,
                )

                # ---- dropout folds (consumed next iteration: full slack)
                nc.gpsimd.tensor_mul(f3next[:], hnext[:], m2_t[:])
                nc.gpsimd.tensor_mul(f2next[0:64, :], rg[0:64, so : so + BC], m1_t[:])

                if tau % 8 == 6:
                    half = (((tau + 1) % 16) - 7) // 8
                    nc.sync.dma_start(
                        h3st[out_row : out_row + 1, :],
                        rg[64:65, half * 8 * BC : (half + 1) * 8 * BC],
                    )
                    out_row += 1

                # boundary fix-ups: wipe garbage states before first real use
                # (L2 steps are real from iter 2, L3 steps from iter 4)
                if tau == 1:
                    nc.vector.memset(Ct[:, 0, :], 0.0)            # C2
                    nc.vector.memset(h2b[0][:], 0.0)              # H2
                if tau == 3:
                    nc.vector.memset(Ct[64:65, 1, :], 0.0)        # C3
                    nc.vector.memset(rg[64:65, 4 * BC : 5 * BC], 0.0)  # H3 slot 4

            # final flush: both halves (tail slots depend on t_steps % 16)
            for half in range(2):
                nc.sync.dma_start(
                    h3st[out_row : out_row + 1, :],
                    rg[64:65, half * 8 * BC : (half + 1) * 8 * BC],
                )
                out_row += 1

    return nc


# ---------------------------------------------------------------- host prep
def pack_weights(Wih1, Whh1, b1, Wih2, Whh2, b2, Wih3, Whh3, b3):
    """Pack/scale weights into the kernel's lhsT layouts (see module doc)."""
    w13 = np.zeros((98, 512), np.float32)
    w2rec = np.zeros((128, 512), np.float32)
    w2fold = np.zeros((65, 512), np.float32)
    w3sk = np.zeros((128, 4), np.float32)
    for s in range(4):
        tg = TG[s]
        gs = 2.0 if s == 3 else 1.0  # sigma(2x) pre-scale for the g slot
        c = s * 128
        # L1 block: rows 0-63 = 2*Whh1^T, row 96 = Wih1, row 97 = b1
        w13[0:64, c : c + 64] = 2.0 * gs * Whh1[tg * 64 : (tg + 1) * 64, :].T
        w13[96, c : c + 64] = gs * Wih1[tg * 64 : (tg + 1) * 64, 0]
        w13[97, c : c + 64] = gs * b1[tg * 64 : (tg + 1) * 64]
        # L3 col 64: row 64 = 2*Whh3, row 97 = b3
        w13[64, c + 64] = 2.0 * gs * Whh3[tg, 0]
        w13[97, c + 64] = gs * b3[tg]
        w3sk[:, s] = 2.0 * gs * Wih3[tg, :]
        # L2
        w2rec[:, c : c + 128] = 2.0 * gs * Whh2[tg * 128 : (tg + 1) * 128, :].T
        w2fold[0:64, c : c + 128] = 2.0 * gs * Wih2[tg * 128 : (tg + 1) * 128, :].T
        w2fold[64, c : c + 128] = gs * b2[tg * 128 : (tg + 1) * 128]
    return dict(w13=w13, w2rec=w2rec, w2fold=w2fold, w3sk=w3sk)


def make_in_maps(inputs, t_steps=T):
    bf = ml_dtypes.bfloat16
    w = pack_weights(
        inputs["Wih1"], inputs["Whh1"], inputs["b1"],
        inputs["Wih2"], inputs["Whh2"], inputs["b2"],
        inputs["Wih3"], inputs["Whh3"], inputs["b3"],
    )
    w = {k: v.astype(bf) for k, v in w.items()}
    x = np.asarray(inputs["x"], np.float32)
    m1 = np.asarray(inputs["mask1"], np.float32)
    m2 = np.asarray(inputs["mask2"], np.float32)
    in_maps = []
    for c in range(NCORES):
        sl = slice(c * BC, (c + 1) * BC)
        nblk = (t_steps + 7) // 8
        xa = np.zeros((nblk, 2, 8 * BC), np.float32)
        xc = x[:t_steps, sl, 0]  # [t_steps, BC]
        for blk in range(nblk):
            n = min(8, t_steps - blk * 8)
            xa[blk, 0, : n * BC] = xc[blk * 8 : blk * 8 + n].reshape(-1)
        xa[:, 1, :] = 1.0
        in_maps.append({
            "xt": xa.astype(bf),
            "m1t": np.ascontiguousarray(m1[sl, :].T).astype(bf),
            "m2t": np.ascontiguousarray(m2[sl, :].T).astype(bf),
            **{k: v for k, v in w.items()},
        })
    return in_maps


def _split_multi_waits(bir):
    """This walrus build allows at most ONE sem wait per instruction.

    Tile's scheduler attaches as many waits as deps require, so split:
    any instruction with k>1 waits gets k-1 single-wait NoOps inserted
    before it on the same engine (sequencer order preserves semantics)."""
    n = 0
    for f in bir.get("functions", []):
        for bb in f.get("basic_blocks", f.get("blocks", [])):
            insts = bb.get("instructions", [])
            out = []
            for inst in insts:
                si = inst.get("sync_info")
                waits = (si or {}).get("on_wait") or []
                if len(waits) > 1:
                    for w in waits[:-1]:
                        n += 1
                        out.append({
                            "debug": inst.get("debug", 0),
                            "engine": inst["engine"],
                            "ins": [],
                            "name": f"WSPLIT-{n}",
                            "opcode": "NoOp",
                            "outs": [],
                            "sync_info": {"on_update": [], "on_wait": [w]},
                            "text_hint": "wait_split",
                        })
                    si["on_wait"] = [waits[-1]]
                out.append(inst)
            bb["instructions"] = out
    return n


def finalize(nc):
    """Apply the multi-wait split to nc's serialized BIR (idempotent)."""
    import orjson

    if getattr(nc, "_wsplit_done", False):
        return nc
    bir = orjson.loads(nc.to_json_bytes())
    n = _split_multi_waits(bir)
    blob = orjson.dumps(bir)
    nc.to_json_bytes = lambda: blob
    nc._wsplit_done = True
    nc._wsplit_count = n
    return nc


def out_schedule(t_steps=T):
    """Replay the out-DMA emission schedule.

    Returns a list (one entry per h3st row) of 8-tuples: the LSTM step
    whose H3 occupies slot j of that row (-1 if junk)."""
    last_write = [None] * 16     # slot -> iter of last H13 write
    rows = []
    for tau in range(t_steps + 4):
        last_write[(tau + 1) % 16] = tau
        if tau % 8 == 6:
            half = (((tau + 1) % 16) - 7) // 8
            rows.append(tuple(
                (last_write[8 * half + j] - 4)
                if last_write[8 * half + j] is not None else -1
                for j in range(8)
            ))
    for half in range(2):
        rows.append(tuple(
            (last_write[8 * half + j] - 4)
            if last_write[8 * half + j] is not None else -1
            for j in range(8)
        ))
    return rows


_BUILT = {}


def kernel(**inputs) -> np.ndarray:
    global LAST_RESULTS
    from concourse.bass_utils import run_bass_kernel_spmd

    if T not in _BUILT:
        _BUILT[T] = finalize(build(T))
    nc = _BUILT[T]
    in_maps = make_in_maps(inputs, T)
    res = run_bass_kernel_spmd(
        nc, in_maps, list(range(NCORES)),
        trace=bool(os.environ.get("BASS_TRACE")),
    )
    LAST_RESULTS = res
    m3 = np.asarray(inputs["mask3"], np.float32)  # [B, 1]
    sched = out_schedule(T)
    out = np.empty((T, B, 1), np.float32)
    for c in range(NCORES):
        sl = slice(c * BC, (c + 1) * BC)
        h3 = np.asarray(res.results[c]["h3st"], dtype=np.float32)  # [n_out, 8*BC]
        dec = np.empty((T, BC), np.float32)
        for r, steps in enumerate(sched):
            for j, st in enumerate(steps):
                if 0 <= st < T:
                    dec[st] = h3[r, j * BC : (j + 1) * BC]
        # h3 = 2*H3; output = h3 * mask3
        out[:, sl, 0] = 2.0 * dec * m3[sl, 0][None, :]
    return out
